# revision 19
# baseline (speedup 1.0000x reference)
# Trainium2 Bass kernel for the EuclideanCodebook (VQ) problem.
#
# Sharding: the 8 codebooks (h axis) map 1:1 onto the 8 NeuronCores.  Every
# core runs an identical program on its own (x[h], embed[h], embed_avg[h],
# cluster_size[h]) slice, so there are no collectives at all — the EMA
# codebook update for codebook h is entirely local to core h.
#
# Per-core program (TOK=16384 tokens, C=1024 codes, D=64):
#   dist[n,c] = 2*x@e^T - |e|^2 - |x|^2 is produced by ONE K=66 matmul:
#     lhsT rows = [x^T (64) ; ones ; |x|^2], rhs rows = [2*e^T ; -|e|^2 ; -1].
#   argmax via DVE max/max_index, onehot via GPSIMD is_ge against the row max,
#   quantize via SWDGE dma_gather (embedding-row gather from HBM),
#   embed_sum/counts via a PSUM-resident [65,1024] accumulating matmul with
#   lhsT = [x | 1] and rhs = onehot.

import os
import numpy as np

H, B, N, D = 8, 8, 2048, 64
C = 1024
TOK = B * N            # 16384 tokens per codebook
PT = 128               # tokens per tile (partition dim)
NT = TOK // PT         # 128 token tiles
BATCH = 16             # token tiles per quantize-gather batch
NB = NT // BATCH
DECAY = 0.8
EPS = 1e-5

_PROGRAM = None
LAST_RESULTS = None


def _build_program():
    import concourse.bacc as bacc
    import concourse.tile as tile
    import concourse.mybir as mybir
    from concourse.masks import make_identity

    fp32 = mybir.dt.float32
    i16 = mybir.dt.int16
    i32 = mybir.dt.int32
    u32 = mybir.dt.uint32
    AF = mybir.ActivationFunctionType
    ALU = mybir.AluOpType

    nc = bacc.Bacc("TRN2", target_bir_lowering=False, debug=False)

    x_h = nc.dram_tensor("x_h", [TOK, D], fp32, kind="ExternalInput")
    embed_h = nc.dram_tensor("embed_h", [C, D], fp32, kind="ExternalInput")
    avg_h = nc.dram_tensor("avg_h", [C, D], fp32, kind="ExternalInput")
    clus_h = nc.dram_tensor("clus_h", [1, C], fp32, kind="ExternalInput")

    dist_h = nc.dram_tensor("dist_h", [TOK, C], fp32, kind="ExternalOutput")
    oneh_h = nc.dram_tensor("oneh_h", [TOK, C], fp32, kind="ExternalOutput")
    quant_h = nc.dram_tensor("quant_h", [TOK, D], fp32, kind="ExternalOutput")
    ind_h = nc.dram_tensor("ind_h", [TOK], i32, kind="ExternalOutput")
    enorm_h = nc.dram_tensor("enorm_h", [C, D], fp32, kind="ExternalOutput")
    eanew_h = nc.dram_tensor("eanew_h", [C, D], fp32, kind="ExternalOutput")
    csnew_h = nc.dram_tensor("csnew_h", [1, C], fp32, kind="ExternalOutput")

    with tile.TileContext(nc) as tc:
        with (
            tc.tile_pool(name="xp", bufs=4) as x_pool,
            tc.tile_pool(name="lhsT", bufs=3) as lhsT_pool,
            tc.tile_pool(name="sq", bufs=2) as sq_pool,
            tc.tile_pool(name="dist", bufs=4) as dist_pool,
            tc.tile_pool(name="oneh", bufs=4) as oneh_pool,
            tc.tile_pool(name="m8", bufs=4) as m8_pool,
            tc.tile_pool(name="wrap", bufs=2) as wrap_pool,
            tc.tile_pool(name="qsb", bufs=2) as q_pool,
            tc.tile_pool(name="ep", bufs=1) as ep_pool,
            tc.tile_pool(name="ptp", bufs=2, space="PSUM") as pt_pool,
            tc.tile_pool(name="pdp", bufs=2, space="PSUM") as pd_pool,
            tc.tile_pool(name="pep", bufs=1, space="PSUM") as pe_pool,
        ):
            # ---------------- prologue (constants / codebook prep) ---------
            ident = ep_pool.tile([128, 128], fp32, tag="ident")
            make_identity(nc, ident[:])
            ones_row = ep_pool.tile([1, 128], fp32, tag="ones_row")
            nc.vector.memset(ones_row[:], 1.0)

            embed_nat = ep_pool.tile([128, 8, D], fp32, tag="embed_nat")
            nc.sync.dma_start(
                embed_nat[:], embed_h[:].rearrange("(j p) d -> p j d", p=128)
            )
            avg_nat = ep_pool.tile([128, 8, D], fp32, tag="avg_nat")
            nc.sync.dma_start(
                avg_nat[:], avg_h[:].rearrange("(j p) d -> p j d", p=128)
            )
            clus_row = ep_pool.tile([1, C], fp32, tag="clus_row")
            nc.sync.dma_start(clus_row[:], clus_h[:])

            # rhs_ext rows: 0 = -|e|^2, 1..64 = 2*embed^T, 65 = -1
            rhs_ext = ep_pool.tile([66, C], fp32, tag="rhs_ext")
            for j in range(8):
                e_ext = sq_pool.tile([128, 66], fp32, tag="e_ext")
                esq = m8_pool.tile([128, 1], fp32, tag="esq")
                e_sq_scr = sq_pool.tile([128, D], fp32, tag="sq")
                nc.scalar.activation(
                    out=e_sq_scr[:], in_=embed_nat[:, j], func=AF.Square,
                    accum_out=esq[:],
                )
                nc.scalar.activation(
                    out=e_ext[:, 1 : D + 1], in_=embed_nat[:, j], func=AF.Copy,
                    scale=2.0,
                )
                nc.vector.tensor_scalar_mul(e_ext[:, 0:1], esq[:], -1.0)
                nc.vector.memset(e_ext[:, D + 1 : D + 2], -1.0)
                ptt = pt_pool.tile([66, 128], fp32, tag="pt")
                nc.tensor.transpose(ptt[:], e_ext[:], ident[:])
                nc.scalar.copy(rhs_ext[:, 128 * j : 128 * (j + 1)], ptt[:])

            # persistent accumulators
            esumT = pe_pool.tile([65, C], fp32, tag="esumT")
            ind16_all = ep_pool.tile([128, NT], i16, tag="ind16_all")
            indf32_all = ep_pool.tile([128, NT], fp32, tag="indf32_all")

            # ------------- main loop over pairs of 128-token tiles ----------
            # One DMA per pair for the x load / dist store / onehot store
            # halves the DMA fixed overhead; onehot stores ride the second
            # HWDGE ring (ACT) so the two big store streams don't share one
            # descriptor generator.
            #
            # The esum matmuls for a pair are emitted one pair LATE: the
            # onehot a tile's esum matmul consumes is ready only ~4us after
            # that tile's dist matmul (PSUM copy -> max -> is_ge), so placing
            # the esum matmuls behind the NEXT pair's dist matmuls in PE
            # program order hides that latency instead of stalling the PE.
            pending_esum = []

            def emit_esum(entries):
                for x_t, oneh_t, t in entries:
                    # esumT[j,c] += sum_n [1|x][n,j] * onehot[n,c]
                    # (row 0 = per-code counts, rows 1..64 = embed_sum^T)
                    nc.tensor.matmul(
                        esumT[:, 0:512], x_t[:, 0 : D + 1], oneh_t[:, 0:512],
                        start=(t == 0), stop=(t == NT - 1),
                    )
                    nc.tensor.matmul(
                        esumT[:, 512:1024], x_t[:, 0 : D + 1],
                        oneh_t[:, 512:1024],
                        start=(t == 0), stop=(t == NT - 1),
                    )

            for tp in range(NT // 2):
                x_w = x_pool.tile([128, 2, 66], fp32, tag="x")
                nc.sync.dma_start(
                    x_w[:, :, 1 : D + 1],
                    x_h[tp * 2 * PT : (tp + 1) * 2 * PT, :]
                    .rearrange("(s p) d -> p s d", p=128),
                )
                nc.gpsimd.memset(x_w[:, :, 0:1], 1.0)
                dist_w = dist_pool.tile([128, 2, C], fp32, tag="dist")
                oneh_w = oneh_pool.tile([128, 2, C], fp32, tag="oneh")
                for s in range(2):
                    t = 2 * tp + s
                    x_t = x_w[:, s]
                    sq_t = sq_pool.tile([128, D], fp32, tag="sq")
                    nc.scalar.activation(
                        out=sq_t[:], in_=x_t[:, 1 : D + 1], func=AF.Square,
                        accum_out=x_t[:, D + 1 : D + 2],
                    )
                    ptt = pt_pool.tile([66, 128], fp32, tag="pt")
                    nc.tensor.transpose(ptt[:], x_t[:], ident[:])
                    lhsT_t = lhsT_pool.tile([66, 128], fp32, tag="lhsT")
                    nc.scalar.copy(lhsT_t[:], ptt[:])

                    pd = pd_pool.tile([128, C], fp32, tag="pd")
                    nc.tensor.matmul(
                        pd[:, 0:512], lhsT_t[:], rhs_ext[:, 0:512],
                        start=True, stop=True,
                    )
                    nc.tensor.matmul(
                        pd[:, 512:1024], lhsT_t[:], rhs_ext[:, 512:1024],
                        start=True, stop=True,
                    )
                    dist_t = dist_w[:, s]
                    nc.scalar.copy(dist_t, pd[:])

                    m8 = m8_pool.tile([128, 8], fp32, tag="m8")
                    nc.vector.max(out=m8[:], in_=dist_t)
                    i8 = m8_pool.tile([128, 8], u32, tag="i8")
                    nc.vector.max_index(out=i8[:], in_max=m8[:], in_values=dist_t)

                    oneh_t = oneh_w[:, s]
                    nc.gpsimd.tensor_scalar(
                        oneh_t, dist_t, m8[:, 0:1], None, op0=ALU.is_ge
                    )
                    pending_esum.append((x_t, oneh_t, t))

                    nc.vector.tensor_copy(ind16_all[:, t : t + 1], i8[:, 0:1])

                if len(pending_esum) > 2:
                    emit_esum(pending_esum[:-2])
                    pending_esum = pending_esum[-2:]

                nc.sync.dma_start(
                    dist_h[tp * 2 * PT : (tp + 1) * 2 * PT, :]
                    .rearrange("(s p) c -> p s c", p=128),
                    dist_w[:],
                )
                nc.scalar.dma_start(
                    oneh_h[tp * 2 * PT : (tp + 1) * 2 * PT, :]
                    .rearrange("(s p) c -> p s c", p=128),
                    oneh_w[:],
                )

                t = 2 * tp + 1
                if t % BATCH == BATCH - 1:
                    b = t // BATCH
                    # Reshuffle indices into dma_gather's wrapped layout:
                    # idx k -> (partition k%16, column k//16).  Token
                    # k = 128*tl + (16*u + q) lives at ind16_all[16u+q, 16b+tl].
                    wrap_b = wrap_pool.tile([128, 8 * BATCH], i16, tag="wrap")
                    for u in range(8):
                        nc.sync.dma_start(
                            wrap_b[0:16, u : 8 * BATCH : 8],
                            ind16_all[16 * u : 16 * (u + 1),
                                      b * BATCH : (b + 1) * BATCH],
                        )
                    # replicate the 16-partition wrapped idxs to all 8 Q7
                    # core slices by doubling
                    for rep in (16, 32, 64):
                        nc.sync.dma_start(
                            wrap_b[rep : 2 * rep, :], wrap_b[0:rep, :]
                        )
                    q_sb = q_pool.tile([128, BATCH, D], fp32, tag="q")
                    nc.gpsimd.dma_gather(
                        q_sb[:], embed_h[:], wrap_b[:],
                        PT * BATCH, PT * BATCH, D,
                        single_packet=False,
                    )
                    nc.sync.dma_start(
                        quant_h[b * BATCH * PT : (b + 1) * BATCH * PT, :]
                        .rearrange("(t p) d -> p t d", p=128),
                        q_sb[:],
                    )

            emit_esum(pending_esum)
            pending_esum = []

            # ---------------- epilogue (EMA codebook update) ----------------
            # f32 view of the collected indices (exact for values < 2^24)
            nc.vector.tensor_copy(indf32_all[:], ind16_all[:])
            # embed_ind: transpose [p, t] -> [t, p] then store int32
            pti = pt_pool.tile([128, 128], fp32, tag="pt")
            nc.tensor.transpose(pti[:], indf32_all[:], ident[:])
            ind_sb = ep_pool.tile([128, 128], i32, tag="ind_sb")
            nc.vector.tensor_copy(ind_sb[:], pti[:])
            nc.sync.dma_start(ind_h[:].rearrange("(i p) -> i p", p=128), ind_sb[:])

            # esumT_s = 0.2 * [counts ; embed_sum^T]
            esumT_s = ep_pool.tile([65, C], fp32, tag="esumT_s")
            nc.scalar.activation(
                out=esumT_s[:], in_=esumT[:], func=AF.Copy, scale=1.0 - DECAY
            )
            cs_tmp = ep_pool.tile([1, C], fp32, tag="cs_tmp")
            nc.vector.tensor_scalar_mul(cs_tmp[:], clus_row[:], DECAY)
            cs_new = ep_pool.tile([1, C], fp32, tag="cs_new")
            nc.vector.tensor_add(cs_new[:], cs_tmp[:], esumT_s[0:1, :])
            nc.sync.dma_start(csnew_h[:], cs_new[:])

            ntot = ep_pool.tile([1, 1], fp32, tag="ntot")
            nc.vector.reduce_sum(ntot[:], cs_new[:], axis=mybir.AxisListType.X)
            den = ep_pool.tile([1, 1], fp32, tag="den")
            nc.vector.tensor_scalar_add(den[:], ntot[:], C * EPS)
            rden = ep_pool.tile([1, 1], fp32, tag="rden")
            nc.vector.reciprocal(rden[:], den[:])
            fac = ep_pool.tile([1, 1], fp32, tag="fac")
            nc.vector.tensor_mul(fac[:], rden[:], ntot[:])
            # broadcast fac to 128 partitions via a K=1 matmul
            pfac = pt_pool.tile([128, 1], fp32, tag="pt")
            nc.tensor.matmul(pfac[:], ones_row[:], fac[:], start=True, stop=True)
            fac_pt = ep_pool.tile([128, 1], fp32, tag="fac_pt")
            nc.vector.tensor_copy(fac_pt[:], pfac[:])
            efac_pt = ep_pool.tile([128, 1], fp32, tag="efac_pt")
            nc.vector.tensor_scalar_mul(efac_pt[:], fac_pt[:], EPS)

            # cs_new to partition layout [128, 8] via 8 tiny PE transposes
            pcs = pt_pool.tile([128, 8], fp32, tag="pt")
            for j in range(8):
                nc.tensor.transpose(
                    pcs[:, j : j + 1],
                    cs_new[0:1, 128 * j : 128 * (j + 1)],
                    ident[0:1, 0:1],
                )
            # cs_smooth = (cs_new + EPS) * ntot/(ntot + C*EPS)
            cs_sm = ep_pool.tile([128, 8], fp32, tag="cs_sm")
            nc.scalar.activation(
                out=cs_sm[:], in_=pcs[:], func=AF.Identity,
                bias=efac_pt[:], scale=fac_pt[:],
            )
            rcs = ep_pool.tile([128, 8], fp32, tag="rcs")
            nc.vector.reciprocal(rcs[:], cs_sm[:])

            # transpose esumT_s back to natural [c, d] layout (8 chunks)
            esum_nat = ep_pool.tile([128, 8, 65], fp32, tag="esum_nat")
            for j in range(8):
                ptj = pt_pool.tile([128, 65], fp32, tag="pt")
                nc.tensor.transpose(
                    ptj[:], esumT_s[:, 128 * j : 128 * (j + 1)], ident[0:65, 0:65]
                )
                nc.scalar.copy(esum_nat[:, j, :], ptj[:])

            avg08 = ep_pool.tile([128, 8, D], fp32, tag="avg08")
            nc.vector.tensor_scalar_mul(avg08[:], avg_nat[:], DECAY)
            ea_nat = ep_pool.tile([128, 8, D], fp32, tag="ea_nat")
            nc.vector.tensor_add(ea_nat[:], avg08[:], esum_nat[:, :, 1 : D + 1])
            nc.sync.dma_start(
                eanew_h[:].rearrange("(j p) d -> p j d", p=128), ea_nat[:]
            )
            en_nat = ep_pool.tile([128, 8, D], fp32, tag="en_nat")
            for j in range(8):
                nc.scalar.activation(
                    out=en_nat[:, j, :], in_=ea_nat[:, j, :], func=AF.Copy,
                    scale=rcs[:, j : j + 1],
                )
            nc.sync.dma_start(
                enorm_h[:].rearrange("(j p) d -> p j d", p=128), en_nat[:]
            )

    nc.compile()
    return nc


def _get_program():
    global _PROGRAM
    if _PROGRAM is None:
        _PROGRAM = _build_program()
    return _PROGRAM


def _in_map_for(x, embed, embed_avg, cluster_size, h):
    return {
        "x_h": np.ascontiguousarray(
            np.asarray(x[h], dtype=np.float32).reshape(TOK, D)
        ),
        "embed_h": np.ascontiguousarray(np.asarray(embed[h], dtype=np.float32)),
        "avg_h": np.ascontiguousarray(np.asarray(embed_avg[h], dtype=np.float32)),
        "clus_h": np.ascontiguousarray(
            np.asarray(cluster_size[h], dtype=np.float32).reshape(1, C)
        ),
    }


def kernel(x, embed, embed_avg, cluster_size):
    global LAST_RESULTS
    from concourse import bass_utils

    nc = _get_program()
    in_maps = [_in_map_for(x, embed, embed_avg, cluster_size, h) for h in range(H)]
    res = bass_utils.run_bass_kernel_spmd(
        nc,
        in_maps,
        core_ids=list(range(H)),
        trace=bool(int(os.environ.get("VQ_TRACE", "0"))),
    )
    LAST_RESULTS = res
    outs = res.results

    quantize = np.stack([outs[h]["quant_h"] for h in range(H)]).reshape(H, B, N, D)
    embed_ind = (
        np.stack([outs[h]["ind_h"] for h in range(H)]).reshape(H, B, N)
    ).astype(np.int32)
    embed_onehot = np.stack([outs[h]["oneh_h"] for h in range(H)])
    dist_out = np.stack([outs[h]["dist_h"] for h in range(H)]).reshape(H, B, N, C)
    embed_norm = np.stack([outs[h]["enorm_h"] for h in range(H)])
    ea_new = np.stack([outs[h]["eanew_h"] for h in range(H)])
    cs_new = np.stack([outs[h]["csnew_h"] for h in range(H)]).reshape(H, C)

    return (
        quantize,
        embed_ind,
        embed_onehot,
        dist_out,
        embed_norm,
        ea_new,
        cs_new,
    )
